# revision 1
# baseline (speedup 1.0000x reference)
"""Trainium2 Bass kernel for nn_CrossAttention (B=4, C=256, H=W=64).

Per (batch, branch) the computation is an independent cross-attention:
    f = Wf @ other + bf          [32, 4096]
    g = Wg @ own   + bg          [32, 4096]
    h = Wh @ own   + bh          [256, 4096]
    S = f^T @ g                  [4096, 4096]
    att = softmax(S, axis=-1)    (normalize over columns m)
    sa[c, m] = sum_n h[c, n] * att[n, m]
    out = gamma * sa + own

There are B*2 = 8 independent problems -> one per NeuronCore (pure SPMD).

Key algebra: att[n, m] = E[n, m] / Z[n] with E = exp(S - K0), Z = rowsum(E),
so sa[c, m] = sum_n (h^T[n,c]/Z[n]) E[n,m].  E is computed ONCE (single exp
pass), kept SBUF-resident in bf16, with Z obtained for free via the
activation accum_out.  The K0 shift cancels exactly in E/Z and guards fp32
exp overflow.  The f/g/h compute path runs in fp16 (same 10-bit mantissa
class as fp32r but full PE rate); E and h/Z use bf16 for exponent range.

Schedule: one slot per exp chunk (n-tile x m-half).  Each slot emits
[trailing work, S-pack, exp]: the trailing work is one sa m-block lagging
SA_SHIFT slots behind the exp pipeline (or, in the first slots, the
remaining conv work), so the in-order PE always has dense work while ACT
streams exps and HAM stays at K=8/8.
"""

import os
import sys

for _p in ("/opt/trn_rl_repo", "/opt/pypackages"):
    if _p not in sys.path:
        sys.path.insert(0, _p)

os.environ.setdefault("JAX_PLATFORMS", "")

import numpy as np

import concourse.bacc as bacc
import concourse.tile as tile
from concourse import mybir

F32 = mybir.dt.float32
F16 = mybir.dt.float16
BF16 = mybir.dt.bfloat16
AF = mybir.ActivationFunctionType

B, C, H, W = 4, 256, 64, 64
N = H * W            # 4096 pixels
C8 = C // 8          # 32
NT = N // 128        # 32 n-tiles
NGROUP = 4           # n-tiles per pipeline group
NG = NT // NGROUP    # 8 groups
MB = 512             # m-block (one PSUM bank of fp32)
NMB = N // MB        # 8 m-blocks
HALF = 2048          # exp chunk (4 PSUM banks)
K0 = 40.0            # constant subtracted inside exp (cancels in softmax)
ICH = 512            # input DMA chunk columns
E_BUFS = 16          # rotating [128, 2048] bf16 E half-tiles
SA_SHIFT = 10        # sa trails the exp pipeline by this many slots


def build_bass():
    nc = bacc.Bacc()

    own_d = nc.dram_tensor("own16", [C, N], F16, kind="ExternalInput")
    oth_d = nc.dram_tensor("oth16", [C, N], F16, kind="ExternalInput")
    res_d = nc.dram_tensor("own32", [C, N], F32, kind="ExternalInput")
    wf_d = nc.dram_tensor("wf_t", [C, C8], F16, kind="ExternalInput")
    wg_d = nc.dram_tensor("wg_t", [C, C8], F16, kind="ExternalInput")
    wh_d = nc.dram_tensor("wh_t", [C, C], F16, kind="ExternalInput")
    bf_d = nc.dram_tensor("bf_rep", [128, 1], F32, kind="ExternalInput")
    bg_d = nc.dram_tensor("bg_rep", [128, 1], F32, kind="ExternalInput")
    bh_d = nc.dram_tensor("bh_row", [1, C], F16, kind="ExternalInput")
    gm_d = nc.dram_tensor("gamma_rep", [128, 1], F32, kind="ExternalInput")
    on_d = nc.dram_tensor("ones_row", [1, 128], F16, kind="ExternalInput")
    k0_d = nc.dram_tensor("k0_col", [128, 1], F32, kind="ExternalInput")
    out_d = nc.dram_tensor("out", [C, N], F32, kind="ExternalOutput")

    NCH = N // ICH  # input chunks per partition-half

    with tile.TileContext(nc) as tc:
        with (
            tc.tile_pool(name="singles", bufs=1) as singles,
            tc.tile_pool(name="inp", bufs=1) as inp,
            tc.tile_pool(name="hxzp", bufs=NT) as hxzp,
            tc.tile_pool(name="epool", bufs=E_BUFS) as epool,
            tc.tile_pool(name="zpool", bufs=4) as zpool,
            tc.tile_pool(name="resp", bufs=3) as resp,
            tc.tile_pool(name="outp", bufs=4) as outp,
            tc.tile_pool(name="ps_fg", bufs=1, space="PSUM") as ps_fg,
            tc.tile_pool(name="ps_h", bufs=1, space="PSUM") as ps_h,
            tc.tile_pool(name="ps_s", bufs=1, space="PSUM") as ps_s,
            tc.tile_pool(name="ps_sa", bufs=2, space="PSUM") as ps_sa,
        ):
            # ---- small constants ----
            wf_sb = [singles.tile([128, C8], F16, name=f"wf{k}") for k in range(2)]
            wg_sb = [singles.tile([128, C8], F16, name=f"wg{k}") for k in range(2)]
            wh_sb = [singles.tile([128, C], F16, name=f"wh{k}") for k in range(2)]
            for k in range(2):
                nc.sync.dma_start(out=wf_sb[k], in_=wf_d[128 * k:128 * (k + 1), :])
                nc.sync.dma_start(out=wg_sb[k], in_=wg_d[128 * k:128 * (k + 1), :])
                nc.sync.dma_start(out=wh_sb[k], in_=wh_d[128 * k:128 * (k + 1), :])
            bf_sb = singles.tile([128, 1], F32)
            bg_sb = singles.tile([128, 1], F32)
            bh_sb = singles.tile([1, C], F16)
            gm_sb = singles.tile([128, 1], F32)
            ones_sb = singles.tile([1, 128], F16)
            k0_sb = singles.tile([128, 1], F32)
            nc.sync.dma_start(out=bf_sb, in_=bf_d[:, :])
            nc.sync.dma_start(out=bg_sb, in_=bg_d[:, :])
            nc.sync.dma_start(out=bh_sb, in_=bh_d[:, :])
            nc.sync.dma_start(out=gm_sb, in_=gm_d[:, :])
            nc.sync.dma_start(out=ones_sb, in_=on_d[:, :])
            nc.sync.dma_start(out=k0_sb, in_=k0_d[:, :])

            # chunked input loads: own/oth [part-half k][chunk c]
            own_sb = [[inp.tile([128, ICH], F16, name=f"own{k}_{c}")
                       for c in range(NCH)] for k in range(2)]
            oth_sb = [[inp.tile([128, ICH], F16, name=f"oth{k}_{c}")
                       for c in range(NCH)] for k in range(2)]
            for c in range(NCH):
                for k in range(2):
                    nc.sync.dma_start(
                        out=own_sb[k][c],
                        in_=own_d[128 * k:128 * (k + 1), ICH * c:ICH * (c + 1)])
                    nc.sync.dma_start(
                        out=oth_sb[k][c],
                        in_=oth_d[128 * k:128 * (k + 1), ICH * c:ICH * (c + 1)])

            # f/g as per-m-block tiles (dependency granularity lets group 0's
            # stats overlap the conv tail); 4 partition-group replicas each.
            f_q = [singles.tile([128, MB], F16, name=f"f{nb}") for nb in range(NMB)]
            g_q = [singles.tile([128, MB], F16, name=f"g{nb}") for nb in range(NMB)]
            sa_sb = [singles.tile([128, N], F32, name=f"sa{k}") for k in range(2)]
            hxz = [hxzp.tile([128, C], BF16, name=f"hxz{i}", tag="hxz")
                   for i in range(NT)]

            def conv_fg(dst, w_sb, src, b_sb, nb):
                ps = ps_fg.tile([128, MB], F32, tag="fg")
                for k in range(2):
                    nc.tensor.matmul(
                        out=ps[0:C8, :],
                        lhsT=w_sb[k],
                        rhs=src[k][nb],
                        start=(k == 0),
                        stop=(k == 1),
                    )
                nc.vector.tensor_scalar(
                    out=dst[nb][0:C8, :],
                    in0=ps[0:C8, :],
                    scalar1=b_sb[0:C8, 0:1],
                    scalar2=None,
                    op0=mybir.AluOpType.add,
                )
                for j in range(1, 4):
                    nc.sync.dma_start(out=dst[nb][32 * j:32 * (j + 1), :],
                                      in_=dst[nb][0:C8, :])

            def conv_h(i):
                c, o = (128 * i) // ICH, (128 * i) % ICH
                ph = ps_h.tile([128, C], F32, tag="h")
                nc.tensor.matmul(out=ph, lhsT=ones_sb, rhs=bh_sb,
                                 start=True, stop=False)
                for k in range(2):
                    nc.tensor.matmul(
                        out=ph,
                        lhsT=own_sb[k][c][:, o:o + 128],
                        rhs=wh_sb[k],
                        start=False,
                        stop=(k == 1),
                    )
                nc.vector.tensor_copy(out=hxz[i], in_=ph)

            # E half-tiles for the in-flight groups: e_half[g % 3][a][h]
            e_half = [[[None] * 2 for _ in range(NGROUP)] for _ in range(3)]

            def stats_chunk(g, a, h, zp):
                """S chunk (n-tile 4g+a, m half h) -> exp -> E + Z part."""
                i = NGROUP * g + a
                nb, o = i // NGROUP, 128 * (i % NGROUP)
                sp = ps_s.tile([128, HALF], F32, tag="s")
                for j in range(4):
                    nc.tensor.matmul(
                        out=sp[:, MB * j:MB * (j + 1)],
                        lhsT=f_q[nb][32 * j:32 * (j + 1), o:o + 128],
                        rhs=g_q[4 * h + j][32 * j:32 * (j + 1), :],
                        start=True,
                        stop=True,
                        tile_position=(32 * j, 0),
                    )
                et = epool.tile([128, HALF], BF16, name=f"e{g}_{a}_{h}", tag="e")
                e_half[g % 3][a][h] = et
                nc.scalar.activation(
                    out=et,
                    in_=sp,
                    func=AF.Exp,
                    bias=k0_sb[:, 0:1],
                    accum_out=zp[:, 2 * a + h:2 * a + h + 1],
                )

            def zprep(g, zp):
                """Z = sum of the two half-sums; hxz *= 1/Z (in place)."""
                zt = zpool.tile([128, NGROUP], F32, tag="zt")
                rz = zpool.tile([128, NGROUP], F32, tag="rz")
                nc.vector.tensor_add(out=zt, in0=zp[:, 0:8:2], in1=zp[:, 1:8:2])
                nc.vector.reciprocal(out=rz, in_=zt)
                for a in range(NGROUP):
                    nc.vector.tensor_scalar(
                        out=hxz[NGROUP * g + a],
                        in0=hxz[NGROUP * g + a],
                        scalar1=rz[:, a:a + 1],
                        scalar2=None,
                        op0=mybir.AluOpType.mult,
                    )

            def sa_mb(g, mb):
                """Accumulate group g's contribution to sa[:, mb block]."""
                h = mb // (NMB // 2)
                m0 = MB * mb - HALF * h
                for ch in range(2):
                    pa = ps_sa.tile([128, MB], F32, tag="sa")
                    for a in range(NGROUP):
                        nc.tensor.matmul(
                            out=pa,
                            lhsT=hxz[NGROUP * g + a][:, 128 * ch:128 * (ch + 1)],
                            rhs=e_half[g % 3][a][h][:, m0:m0 + MB],
                            start=(a == 0),
                            stop=(a == NGROUP - 1),
                        )
                    dst = sa_sb[ch][:, MB * mb:MB * (mb + 1)]
                    if g == 0:
                        nc.vector.tensor_copy(out=dst, in_=pa)
                    else:
                        nc.vector.tensor_add(out=dst, in0=dst, in1=pa)

            def epilogue_mb(mb):
                for ch in range(2):
                    xr = resp.tile([128, MB], F32, tag="xr")
                    nc.sync.dma_start(
                        out=xr,
                        in_=res_d[128 * ch:128 * (ch + 1), MB * mb:MB * (mb + 1)])
                    ot = outp.tile([128, MB], F32, tag="ot")
                    nc.vector.scalar_tensor_tensor(
                        out=ot,
                        in0=sa_sb[ch][:, MB * mb:MB * (mb + 1)],
                        scalar=gm_sb[:, 0:1],
                        in1=xr,
                        op0=mybir.AluOpType.mult,
                        op1=mybir.AluOpType.add,
                    )
                    nc.sync.dma_start(
                        out=out_d[128 * ch:128 * (ch + 1), MB * mb:MB * (mb + 1)],
                        in_=ot,
                    )

            # ---- slot schedule ----
            # Conv work not needed before the first stats chunk becomes the
            # trailing filler for the first SA_SHIFT slots; afterwards the
            # trailing sa m-blocks (lagging SA_SHIFT slots) fill that role.
            filler = [("g", nb) for nb in range(4, NMB)] \
                   + [("f", nb) for nb in range(1, NMB)] \
                   + [("h", i) for i in range(NT)]
            # upfront: everything the first stats chunk (n-tile 0, half 0)
            # needs: f block 0 and g blocks 0..3.
            conv_fg(f_q, wf_sb, oth_sb, bf_sb, 0)
            for nb in range(4):
                conv_fg(g_q, wg_sb, own_sb, bg_sb, nb)

            fill_per_slot = (len(filler) + SA_SHIFT - 1) // SA_SHIFT
            zps = {}

            def emit_slot_filler(pos):
                sidx = pos - SA_SHIFT
                if sidx >= 0:
                    sg, smb = sidx // NMB, sidx % NMB
                    if smb == 0:
                        zprep(sg, zps.pop(sg))
                    sa_mb(sg, smb)
                    if sg == NG - 1:
                        epilogue_mb(smb)
                else:
                    for _ in range(fill_per_slot):
                        if filler:
                            kind, arg = filler.pop(0)
                            if kind == "g":
                                conv_fg(g_q, wg_sb, own_sb, bg_sb, arg)
                            elif kind == "f":
                                conv_fg(f_q, wf_sb, oth_sb, bf_sb, arg)
                            else:
                                conv_h(arg)

            chunks = [(a, h) for h in range(2) for a in range(NGROUP)]
            for g in range(NG):
                zps[g] = zpool.tile([128, 2 * NGROUP], F32, tag="zp", name=f"zp{g}")
                for k, (a, h) in enumerate(chunks):
                    emit_slot_filler(g * 8 + k)
                    stats_chunk(g, a, h, zps[g])
            for pos in range(NG * 8, NG * 8 + SA_SHIFT):
                emit_slot_filler(pos)

    # run_bass_via_pjrt binds the exec primitive directly and never
    # finalizes; Bacc's register allocation + matmul-wait splitting live in
    # finalize()/compile(), so run it here.
    if not nc.is_finalized():
        nc.finalize()
    return nc


_NC_CACHE = None


def _get_nc():
    global _NC_CACHE
    if _NC_CACHE is None:
        _NC_CACHE = build_bass()
    return _NC_CACHE


def make_in_maps(**inputs):
    """Build the 8 per-core input maps (core 2b = x-branch, 2b+1 = y-branch)."""
    f = lambda a: np.ascontiguousarray(np.asarray(a), dtype=np.float32)
    h16 = lambda a: np.ascontiguousarray(np.asarray(a), dtype=np.float16)
    x = f(inputs["x"]).reshape(B, C, N)
    y = f(inputs["y"]).reshape(B, C, N)
    x16, y16 = x.astype(np.float16), y.astype(np.float16)
    Wfx, bfx = h16(inputs["Wfx"]), f(inputs["bfx"])
    Wgx, bgx = h16(inputs["Wgx"]), f(inputs["bgx"])
    Whx, bhx = h16(inputs["Whx"]), h16(inputs["bhx"])
    Wfy, bfy = h16(inputs["Wfy"]), f(inputs["bfy"])
    Wgy, bgy = h16(inputs["Wgy"]), f(inputs["bgy"])
    Why, bhy = h16(inputs["Why"]), h16(inputs["bhy"])
    gamma = f(inputs["gamma"])

    rep4 = lambda b: np.ascontiguousarray(np.tile(b, 4).reshape(128, 1))
    gam = np.ascontiguousarray(np.broadcast_to(gamma.reshape(1, 1), (128, 1)))

    c16 = lambda a: np.ascontiguousarray(a, dtype=np.float16)
    branch = {
        "x": dict(
            wf_t=c16(Wfy.T), wg_t=c16(Wgx.T), wh_t=c16(Whx.T),
            bf_rep=rep4(bfy), bg_rep=rep4(bgx), bh_row=c16(bhx.reshape(1, C)),
        ),
        "y": dict(
            wf_t=c16(Wfx.T), wg_t=c16(Wgy.T), wh_t=c16(Why.T),
            bf_rep=rep4(bfx), bg_rep=rep4(bgy), bh_row=c16(bhy.reshape(1, C)),
        ),
    }

    ones_row = np.ones((1, 128), np.float16)
    k0_col = np.full((128, 1), -K0, np.float32)
    in_maps = []
    for b in range(B):
        in_maps.append(dict(own16=x16[b], oth16=y16[b], own32=x[b],
                            gamma_rep=gam, ones_row=ones_row, k0_col=k0_col,
                            **branch["x"]))
        in_maps.append(dict(own16=y16[b], oth16=x16[b], own32=y[b],
                            gamma_rep=gam, ones_row=ones_row, k0_col=k0_col,
                            **branch["y"]))
    return in_maps


def kernel(**inputs):
    from concourse.bass_utils import run_bass_kernel_spmd

    nc = _get_nc()
    in_maps = make_in_maps(**inputs)
    res = run_bass_kernel_spmd(nc, in_maps, list(range(8))).results
    out_x = np.stack([res[2 * b]["out"] for b in range(B)]).reshape(B, C, H, W)
    out_y = np.stack([res[2 * b + 1]["out"] for b in range(B)]).reshape(B, C, H, W)
    return (out_x, out_y)

